# revision 6
# baseline (speedup 1.0000x reference)
"""Trainium2 Bass kernel: CNN encoder (conv1d F=8, D=128 -> K=256, valid, + bias + ReLU).

Computation: out[b, l, k] = relu(b_k[k] + sum_{f,d} x[b, l+f, d] * filt[f,d] * W[f*D+d, k])
for l in [0, L-F)  (2040 windows).

Strategy:
  - Data-parallel: 32 batches / 8 cores = 4 batches per core. Params replicated.
  - Host folds filt into W (Wp[f,d,k] = filt[f,d]*W[f*128+d,k]) and transposes x to
    d-major (xT[b, d, l]) so the contraction dim (d=128) lands on SBUF partitions
    with fully-contiguous DMA.
  - Operands in bf16 (quantization rel-err ~2.4e-3, well under the 2e-2 gate):
    matmul still streams 1 cycle/row but weight loads get the hardware fast-weight-
    load (FWL) path, which fp32 dtypes disable.
  - On device, per (batch, k-half): all four 512-wide output stripes accumulate in
    four PSUM banks simultaneously, with the tap loop OUTERMOST:
        for f in 0..7: for stripe s in 0..3: psum[s] += Wp[f,:,kh].T @ xT[:, l0s+f:+512]
    Four consecutive matmuls share the same stationary weights, so the PE issues
    one LDWEIGHTS per 4 matmuls instead of per matmul (the baseline's serialized
    per-matmul fp32r loads cost 107ns against a 213ns matmul).
  - (b, kh) units ping-pong across the two PSUM bank quads (8 banks total) so
    eviction of one quad overlaps accumulation in the other.
  - Eviction fuses bias-add + ReLU (bias is per-partition since k is the partition
    dim), alternating ScalarE activation / VectorE tensor_scalar.
  - Output written k-major ([b, k, l]); host transposes back to [b, l, k].
"""

import os

import numpy as np

import concourse.bacc as bacc
import concourse.bass as bass
import concourse.tile as tile
import concourse.mybir as mybir
from concourse.bass_utils import run_bass_kernel_spmd

if os.environ.get("LDW_OPT") == "1":
    # experiment: let walrus optimize the per-matmul fused weight loads
    from concourse import bass_utils as _bu
    if not getattr(_bu, "_ldw_opt_patched", False):
        _orig_run_command = _bu.run_command

        def _patched_run_command(argv, **kw):
            argv = ["--enable-ldw-opt=true" if a == "--enable-ldw-opt=false" else a
                    for a in argv]
            return _orig_run_command(argv, **kw)

        _bu.run_command = _patched_run_command
        _bu._ldw_opt_patched = True

F32 = mybir.dt.float32
F32R = mybir.dt.float32r
BF16 = mybir.dt.bfloat16

N_CORES = 8
B, L, D = 32, 2048, 128
F, K = 8, 256
N_WIN = L - F            # 2040
BP = B // N_CORES        # batches per core
KH = K // 128            # k halves
# output-position stripes per batch: 3x512 + 1x504
SUPERS = [(0, 512), (512, 512), (1024, 512), (1536, N_WIN - 1536)]

# schedule: "fouter" = tap-loop outermost, 4 interleaved PSUM groups per (b,kh)
#           "pairs"  = baseline pair schedule (one group at a time per pair)
SCHED = os.environ.get("SCHED", "fouter")

# operand dtype: bf16 (FWL fast weight loads) or f32r (precise, slow loads)
W_DT = F32R if os.environ.get("WDT", "bf16") == "f32r" else BF16
X_DT = W_DT

PSUM_BUFS = int(os.environ.get("PSUM_BUFS", "8" if SCHED == "fouter" else "6"))

# WARM=0 drops the HAM warm-up block (for looped benching, where the loop
# itself keeps the PE warm and the warm-up would bill ~14us/iteration).
WARM = os.environ.get("WARM", "1") == "1"


def _build_program(reps=1, loop_n=0):
    """One SPMD program for all 8 cores. reps>1 unrolls the full body (input
    DMAs + compute + output DMAs); rep r writes to output rows [r*BP, (r+1)*BP).
    loop_n>0 additionally wraps the body in a hardware For_i loop (benchmarking
    only: every loop iteration rewrites the same output region)."""
    nc = bacc.Bacc(
        "TRN2",
        target_bir_lowering=False,
        debug=False,
        num_devices=N_CORES,
    )
    xT_d = nc.declare_dram_parameter("xT", [BP, D, L], X_DT, isOutput=False)
    wp_d = nc.declare_dram_parameter("wp", [KH, D, F, 128], W_DT, isOutput=False)
    bias_d = nc.declare_dram_parameter("bias", [128, KH], F32, isOutput=False)
    out_d = nc.declare_dram_parameter(
        "outT", [reps * BP, KH, 128, N_WIN], F32, isOutput=True)

    # xt halves: lo covers l in [0, HALF+F), hi covers [HALF, L). Stripes 0-1
    # read only lo, stripes 2-3 only hi, so each matmul depends on exactly one
    # half-tile DMA instead of the full batch load.
    HALF = L // 2
    LO_W = HALF + F  # 1032

    # out-DMA stream points: after stripe si completes, DMA ob columns [lo, hi)
    OUT_CHUNKS = {1: (0, 1024), 2: (1024, 1536), 3: (1536, N_WIN)}

    def body(nc, tc, pools, r, warm):
        const_pool, xt_pool, psum_pool, out_pool = pools
        bias_sb = const_pool.tile([128, KH], F32, tag="bias")
        wp_sb = []
        for kh in range(KH):
            t_wp = const_pool.tile([D, F, 128], W_DT, tag=f"wp{kh}")
            wp_sb.append(t_wp)

        xt_lo, xt_hi = [], []
        for b in range(BP):
            t_lo = xt_pool.tile([D, LO_W], X_DT, tag="xtlo")
            t_hi = xt_pool.tile([D, L - HALF], X_DT, tag="xthi")
            xt_lo.append(t_lo)
            xt_hi.append(t_hi)

        if warm and WARM:
            # PE HAM warm-up on junk data while the first input DMAs land.
            # Plain fp32 matmuls (4 cycles/row, ~850ns each) keep the PE busy
            # through the ~3.5us clock-gate window.
            warm_x = const_pool.tile([D, 64], F32, tag="warmx")
            warm_ps = psum_pool.tile([128, 512], F32, tag="ps")
            nc.gpsimd.memset(warm_x[:], 0.0)
            for _ in range(16):
                nc.tensor.matmul(warm_ps[0:64, 0:64], lhsT=warm_x[:, 0:64],
                                 rhs=warm_x[:], start=True, stop=True)

        # issue order: batch-0 lo + first weight half first so compute starts ASAP
        nc.sync.dma_start(xt_lo[0][:], xT_d[0, :, 0:LO_W])
        nc.sync.dma_start(wp_sb[0][:], wp_d[0])
        nc.sync.dma_start(bias_sb[:], bias_d[:])
        nc.sync.dma_start(wp_sb[1][:], wp_d[1])
        nc.sync.dma_start(xt_hi[0][:], xT_d[0, :, HALF:L])
        for b in range(1, BP):
            nc.sync.dma_start(xt_lo[b][:], xT_d[b, :, 0:LO_W])
            nc.sync.dma_start(xt_hi[b][:], xT_d[b, :, HALF:L])

        evictor = 0

        def evict(ob, ps, si, b, kh):
            nonlocal evictor
            l0, ls = SUPERS[si]
            if evictor == 0:
                nc.scalar.activation(
                    ob[:, l0:l0 + ls], ps[:, :ls],
                    mybir.ActivationFunctionType.Relu,
                    bias=bias_sb[:, kh:kh + 1], scale=1.0,
                )
            else:
                nc.vector.tensor_scalar(
                    ob[:, l0:l0 + ls], ps[:, :ls],
                    scalar1=bias_sb[:, kh:kh + 1], scalar2=0.0,
                    op0=mybir.AluOpType.add, op1=mybir.AluOpType.max,
                )
            evictor ^= 1
            if si in OUT_CHUNKS:
                lo, hi = OUT_CHUNKS[si]
                nc.sync.dma_start(out_d[r * BP + b, kh, :, lo:hi],
                                  ob[:, lo:hi])

        if SCHED == "fouter":
            # Tap loop outermost: the four stripes of a (b,kh) unit accumulate
            # in four PSUM banks at once, so four consecutive matmuls share one
            # stationary weight slice (one LDWEIGHTS per 4 matmuls).
            for b in range(BP):
                for kh in range(KH):
                    ob = out_pool.tile([128, N_WIN], F32, tag="ob")
                    pss, bases, lss = [], [], []
                    for si in range(4):
                        l0, ls = SUPERS[si]
                        t_ps = psum_pool.tile([128, 512], F32, tag="ps")
                        pss.append(t_ps)
                        bases.append(l0 if si < 2 else l0 - HALF)
                        lss.append(ls)
                    for f in range(F):
                        for si in range(4):
                            xt = xt_lo[b] if si < 2 else xt_hi[b]
                            nc.tensor.matmul(
                                pss[si][:, :lss[si]],
                                lhsT=wp_sb[kh][:, f, :],
                                rhs=xt[:, bases[si] + f:bases[si] + f + lss[si]],
                                start=(f == 0),
                                stop=(f == F - 1),
                            )
                    for si in range(4):
                        evict(ob, pss[si], si, b, kh)
        else:
            # Baseline pair schedule: stripes processed in pairs (s0,s1), (s2,s3);
            # each pair's two accumulation groups run sequentially.
            for b in range(BP):
                for kh in range(KH):
                    ob = out_pool.tile([128, N_WIN], F32, tag="ob")
                    for pair in range(2):
                        xt = xt_lo[b] if pair == 0 else xt_hi[b]
                        sis = (2 * pair, 2 * pair + 1)
                        for si in sis:
                            l0, ls = SUPERS[si]
                            ps = psum_pool.tile([128, 512], F32, tag="ps")
                            base = l0 if pair == 0 else l0 - HALF
                            for f in range(F):
                                nc.tensor.matmul(
                                    ps[:, :ls],
                                    lhsT=wp_sb[kh][:, f, :],
                                    rhs=xt[:, base + f:base + f + ls],
                                    start=(f == 0),
                                    stop=(f == F - 1),
                                )
                            evict(ob, ps, si, b, kh)

    with tile.TileContext(nc) as tc:
        with (
            tc.tile_pool(name="const", bufs=2) as const_pool,
            tc.tile_pool(name="xt", bufs=BP) as xt_pool,
            tc.tile_pool(name="psum", bufs=PSUM_BUFS,
                         space=bass.MemorySpace.PSUM) as psum_pool,
            tc.tile_pool(name="out", bufs=4) as out_pool,
        ):
            pools = (const_pool, xt_pool, psum_pool, out_pool)
            if loop_n > 0:
                with tc.For_i(0, loop_n, 1,
                              hint_engines=(mybir.EngineType.PE,)):
                    for r in range(reps):
                        body(nc, tc, pools, r, warm=(r == 0))
            else:
                for r in range(reps):
                    body(nc, tc, pools, r, warm=(r == 0))
    nc.compile()
    return nc


def _prep_inputs(user_batch, filt, W_k, b_k):
    user_batch = np.asarray(user_batch, dtype=np.float32)
    filt = np.asarray(filt, dtype=np.float32)
    W_k = np.asarray(W_k, dtype=np.float32)
    b_k = np.asarray(b_k, dtype=np.float32)

    wp = W_k.reshape(F, D, K) * filt[:, :, None]          # [f, d, k]
    wp_host = np.ascontiguousarray(                        # [kh, d, f, 128]
        wp.reshape(F, D, KH, 128).transpose(2, 1, 0, 3))
    bias_host = np.ascontiguousarray(b_k.reshape(KH, 128).T)  # [128, kh]
    xT = np.ascontiguousarray(user_batch.transpose(0, 2, 1))  # [b, d, l]
    if W_DT == BF16:
        import ml_dtypes
        wp_host = wp_host.astype(ml_dtypes.bfloat16)
        xT = xT.astype(ml_dtypes.bfloat16)
    return xT, wp_host, bias_host


def _make_in_maps(prepped):
    xT, wp_host, bias_host = prepped
    return [
        {"xT": xT[c * BP:(c + 1) * BP], "wp": wp_host, "bias": bias_host}
        for c in range(N_CORES)
    ]


def _run(user_batch, filt, W_k, b_k, trace=False):
    prepped = _prep_inputs(user_batch, filt, W_k, b_k)
    nc = _build_program()
    in_maps = _make_in_maps(prepped)
    res = run_bass_kernel_spmd(nc, in_maps, list(range(N_CORES)), trace=trace)
    outT = np.concatenate([r["outT"] for r in res.results], axis=0)  # [B, KH, 128, N_WIN]
    out = outT.reshape(B, K, N_WIN).transpose(0, 2, 1)               # [B, N_WIN, K]
    return np.ascontiguousarray(out), res


def kernel(user_batch, filt, W_k, b_k):
    out, _ = _run(user_batch, filt, W_k, b_k, trace=False)
    return out


# revision 15
# speedup vs baseline: 1.3876x; 1.3876x over previous
"""Trainium2 Bass kernel: CNN encoder (conv1d F=8, D=128 -> K=256, valid, + bias + ReLU).

Computation: out[b, l, k] = relu(b_k[k] + sum_{f,d} x[b, l+f, d] * filt[f,d] * W[f*D+d, k])
for l in [0, L-F)  (2040 windows).

Strategy:
  - Data-parallel: 32 batches / 8 cores = 4 batches per core. Params replicated.
  - Host folds filt into W (Wp[f,d,k] = filt[f,d]*W[f*128+d,k]) and transposes x to
    d-major (xT[b, d, l]) so the contraction dim (d=128) lands on SBUF partitions
    with fully-contiguous DMA.
  - Operands in bf16 (quantization rel-err ~2.2e-3, well under the 2e-2 gate):
    matmul still streams 1 cycle/row but weight loads get the hardware fast-weight-
    load (FWL) path, which fp32 dtypes disable (measured: bf16 beats fp32r by
    ~45us/iter in the looped bench). Output is written bf16 too (half the
    output DMA bytes, +2^-9 rounding; host converts back to fp32).
  - On device, per (batch, k-half): all four 512-wide output stripes accumulate in
    four PSUM banks simultaneously, with the tap loop OUTERMOST:
        for f in 0..7: for stripe s in 0..3: psum[s] += Wp[f,:,kh].T @ xT[:, l0s+f:+512]
    Four consecutive matmuls share the same stationary weights, so the PE issues
    one LDWEIGHTS per 4 matmuls instead of per matmul (the baseline's serialized
    per-matmul fp32r loads cost 107ns against a 213ns matmul).
  - (b, kh) units ping-pong across the two PSUM bank quads (8 banks total) so
    eviction of one quad overlaps accumulation in the other.
  - Eviction fuses bias-add + ReLU (bias is per-partition since k is the partition
    dim), alternating ScalarE activation / VectorE tensor_scalar.
  - Output written k-major ([b, k, l]); host transposes back to [b, l, k].
"""

import os

import numpy as np

import concourse.bacc as bacc
import concourse.bass as bass
import concourse.tile as tile
import concourse.mybir as mybir
from concourse.bass_utils import run_bass_kernel_spmd

# NOTE: walrus --enable-ldw-opt=true was tried and crashes codegen
# (visitInstLdweights internal error), so the per-matmul weight loads are
# amortized by instruction ordering instead (see SCHED=fouter below).

F32 = mybir.dt.float32
F32R = mybir.dt.float32r
BF16 = mybir.dt.bfloat16

N_CORES = 8
B, L, D = 32, 2048, 128
F, K = 8, 256
N_WIN = L - F            # 2040
BP = B // N_CORES        # batches per core
KH = K // 128            # k halves
# output-position stripes per batch: 3x512 + 1x504
SUPERS = [(0, 512), (512, 512), (1024, 512), (1536, N_WIN - 1536)]

# schedule: "fouter" = tap-loop outermost, 4 interleaved PSUM groups per (b,kh)
#           "pairs"  = baseline pair schedule (one group at a time per pair)
SCHED = os.environ.get("SCHED", "fouter")

# operand dtype: bf16 (FWL fast weight loads) or f32r (precise, slow loads)
W_DT = F32R if os.environ.get("WDT", "bf16") == "f32r" else BF16
X_DT = W_DT

PSUM_BUFS = int(os.environ.get("PSUM_BUFS", "8" if SCHED == "fouter" else "6"))

# WARM=0 drops the HAM warm-up block (for looped benching, where the loop
# itself keeps the PE warm and the warm-up would bill ~14us/iteration).
WARM = os.environ.get("WARM", "1") == "1"

# Engine whose DGE queue carries the output DMAs. Inputs always go on SP's
# HWDGE queue; putting outputs elsewhere ("pool" SWDGE by default) keeps the
# two streams from serializing on one queue.
OUT_DMA = os.environ.get("OUT_DMA", "pool")

# xt buffers per tag: 2*BP ping-pongs input tiles across loop iterations so
# the next iteration's input DMAs overlap this iteration's compute.
XT_BUFS = int(os.environ.get("XT_BUFS", str(2 * BP)))

# OUT_BF16=1 stores the output tile + DRAM tensor as bf16 (half the output
# DMA bytes; host converts back to fp32). Adds ~2^-9 relative rounding.
OUT_BF16 = os.environ.get("OUT_BF16", "1") == "1"
O_DT = BF16 if OUT_BF16 else F32


def _build_program(reps=1, loop_n=0):
    """One SPMD program for all 8 cores. reps>1 unrolls the full body (input
    DMAs + compute + output DMAs); rep r writes to output rows [r*BP, (r+1)*BP).
    loop_n>0 additionally wraps the body in a hardware For_i loop (benchmarking
    only: every loop iteration rewrites the same output region)."""
    nc = bacc.Bacc(
        "TRN2",
        target_bir_lowering=False,
        debug=False,
        num_devices=N_CORES,
    )
    xT_d = nc.declare_dram_parameter("xT", [BP, D, L], X_DT, isOutput=False)
    wp_d = nc.declare_dram_parameter("wp", [KH, D, F, 128], W_DT, isOutput=False)
    bias_d = nc.declare_dram_parameter("bias", [128, KH], F32, isOutput=False)
    out_d = nc.declare_dram_parameter(
        "outT", [reps * BP, KH, 128, N_WIN], O_DT, isOutput=True)

    # xt halves: lo covers l in [0, HALF+F), hi covers [HALF, L). Stripes 0-1
    # read only lo, stripes 2-3 only hi, so each matmul depends on exactly one
    # half-tile DMA instead of the full batch load.
    HALF = L // 2
    LO_W = HALF + F  # 1032

    # out-DMA stream points: after stripe si completes, DMA ob columns [lo, hi)
    OUT_CHUNKS = {1: (0, 1024), 2: (1024, 1536), 3: (1536, N_WIN)}

    def body(nc, tc, pools, r, warm):
        const_pool, xt_pool, psum_pool, out_pool = pools
        out_dma = {"pool": nc.gpsimd, "act": nc.scalar, "sp": nc.sync}[OUT_DMA]
        bias_sb = const_pool.tile([128, KH], F32, tag="bias")
        wp_sb = []
        for kh in range(KH):
            t_wp = const_pool.tile([D, F, 128], W_DT, tag=f"wp{kh}")
            wp_sb.append(t_wp)

        xt_lo, xt_hi = [], []
        for b in range(BP):
            t_lo = xt_pool.tile([D, LO_W], X_DT, tag="xtlo")
            t_hi = xt_pool.tile([D, L - HALF], X_DT, tag="xthi")
            xt_lo.append(t_lo)
            xt_hi.append(t_hi)

        if warm and WARM:
            # PE HAM warm-up on junk data while the first input DMAs land.
            # Plain fp32 matmuls (4 cycles/row, ~850ns each) keep the PE busy
            # through the ~3.5us clock-gate window.
            warm_x = const_pool.tile([D, 64], F32, tag="warmx")
            warm_ps = psum_pool.tile([128, 512], F32, tag="ps")
            nc.gpsimd.memset(warm_x[:], 0.0)
            for _ in range(16):
                nc.tensor.matmul(warm_ps[0:64, 0:64], lhsT=warm_x[:, 0:64],
                                 rhs=warm_x[:], start=True, stop=True)

        # issue order: batch-0 lo + first weight half first so compute starts ASAP
        nc.sync.dma_start(xt_lo[0][:], xT_d[0, :, 0:LO_W])
        nc.sync.dma_start(wp_sb[0][:], wp_d[0])
        nc.sync.dma_start(bias_sb[:], bias_d[:])
        nc.sync.dma_start(wp_sb[1][:], wp_d[1])
        nc.sync.dma_start(xt_hi[0][:], xT_d[0, :, HALF:L])
        for b in range(1, BP):
            nc.sync.dma_start(xt_lo[b][:], xT_d[b, :, 0:LO_W])
            nc.sync.dma_start(xt_hi[b][:], xT_d[b, :, HALF:L])

        evictor = 0

        def evict(ob, ps, si, b, kh):
            nonlocal evictor
            l0, ls = SUPERS[si]
            if evictor == 0:
                nc.scalar.activation(
                    ob[:, l0:l0 + ls], ps[:, :ls],
                    mybir.ActivationFunctionType.Relu,
                    bias=bias_sb[:, kh:kh + 1], scale=1.0,
                )
            else:
                nc.vector.tensor_scalar(
                    ob[:, l0:l0 + ls], ps[:, :ls],
                    scalar1=bias_sb[:, kh:kh + 1], scalar2=0.0,
                    op0=mybir.AluOpType.add, op1=mybir.AluOpType.max,
                )
            evictor ^= 1
            if si in OUT_CHUNKS:
                lo, hi = OUT_CHUNKS[si]
                out_dma.dma_start(out_d[r * BP + b, kh, :, lo:hi],
                                  ob[:, lo:hi])

        if SCHED == "fouter":
            # Tap loop outermost: the four stripes of a (b,kh) unit accumulate
            # in four PSUM banks at once, so four consecutive matmuls share one
            # stationary weight slice (one LDWEIGHTS per 4 matmuls).
            for b in range(BP):
                for kh in range(KH):
                    ob = out_pool.tile([128, N_WIN], O_DT, tag="ob")
                    pss, bases, lss = [], [], []
                    for si in range(4):
                        l0, ls = SUPERS[si]
                        t_ps = psum_pool.tile([128, 512], F32, tag="ps")
                        pss.append(t_ps)
                        bases.append(l0 if si < 2 else l0 - HALF)
                        lss.append(ls)
                    for f in range(F):
                        for si in range(4):
                            xt = xt_lo[b] if si < 2 else xt_hi[b]
                            nc.tensor.matmul(
                                pss[si][:, :lss[si]],
                                lhsT=wp_sb[kh][:, f, :],
                                rhs=xt[:, bases[si] + f:bases[si] + f + lss[si]],
                                start=(f == 0),
                                stop=(f == F - 1),
                            )
                    for si in range(4):
                        evict(ob, pss[si], si, b, kh)
        else:
            # Baseline pair schedule: stripes processed in pairs (s0,s1), (s2,s3);
            # each pair's two accumulation groups run sequentially.
            for b in range(BP):
                for kh in range(KH):
                    ob = out_pool.tile([128, N_WIN], O_DT, tag="ob")
                    for pair in range(2):
                        xt = xt_lo[b] if pair == 0 else xt_hi[b]
                        sis = (2 * pair, 2 * pair + 1)
                        for si in sis:
                            l0, ls = SUPERS[si]
                            ps = psum_pool.tile([128, 512], F32, tag="ps")
                            base = l0 if pair == 0 else l0 - HALF
                            for f in range(F):
                                nc.tensor.matmul(
                                    ps[:, :ls],
                                    lhsT=wp_sb[kh][:, f, :],
                                    rhs=xt[:, base + f:base + f + ls],
                                    start=(f == 0),
                                    stop=(f == F - 1),
                                )
                            evict(ob, ps, si, b, kh)

    with tile.TileContext(nc) as tc:
        with (
            tc.tile_pool(name="const", bufs=2) as const_pool,
            tc.tile_pool(name="xt", bufs=XT_BUFS) as xt_pool,
            tc.tile_pool(name="psum", bufs=PSUM_BUFS,
                         space=bass.MemorySpace.PSUM) as psum_pool,
            tc.tile_pool(name="out", bufs=4) as out_pool,
        ):
            pools = (const_pool, xt_pool, psum_pool, out_pool)
            if loop_n > 0:
                with tc.For_i(0, loop_n, 1,
                              hint_engines=(mybir.EngineType.PE,)):
                    for r in range(reps):
                        body(nc, tc, pools, r, warm=(r == 0))
            else:
                for r in range(reps):
                    body(nc, tc, pools, r, warm=(r == 0))
    nc.compile()
    return nc


def _prep_inputs(user_batch, filt, W_k, b_k):
    user_batch = np.asarray(user_batch, dtype=np.float32)
    filt = np.asarray(filt, dtype=np.float32)
    W_k = np.asarray(W_k, dtype=np.float32)
    b_k = np.asarray(b_k, dtype=np.float32)

    wp = W_k.reshape(F, D, K) * filt[:, :, None]          # [f, d, k]
    wp_host = np.ascontiguousarray(                        # [kh, d, f, 128]
        wp.reshape(F, D, KH, 128).transpose(2, 1, 0, 3))
    bias_host = np.ascontiguousarray(b_k.reshape(KH, 128).T)  # [128, kh]
    xT = np.ascontiguousarray(user_batch.transpose(0, 2, 1))  # [b, d, l]
    if W_DT == BF16:
        import ml_dtypes
        wp_host = wp_host.astype(ml_dtypes.bfloat16)
        xT = xT.astype(ml_dtypes.bfloat16)
    return xT, wp_host, bias_host


def _make_in_maps(prepped):
    xT, wp_host, bias_host = prepped
    return [
        {"xT": xT[c * BP:(c + 1) * BP], "wp": wp_host, "bias": bias_host}
        for c in range(N_CORES)
    ]


def _run(user_batch, filt, W_k, b_k, trace=False):
    prepped = _prep_inputs(user_batch, filt, W_k, b_k)
    nc = _build_program()
    in_maps = _make_in_maps(prepped)
    res = run_bass_kernel_spmd(nc, in_maps, list(range(N_CORES)), trace=trace)
    outT = np.concatenate([r["outT"] for r in res.results], axis=0)  # [B, KH, 128, N_WIN]
    outT = outT.astype(np.float32)
    out = outT.reshape(B, K, N_WIN).transpose(0, 2, 1)               # [B, N_WIN, K]
    return np.ascontiguousarray(out), res


def kernel(user_batch, filt, W_k, b_k):
    out, _ = _run(user_batch, filt, W_k, b_k, trace=False)
    return out
